# revision 30
# baseline (speedup 1.0000x reference)
"""Trainium2 Bass kernel for nn_ECA_69544110457542.

Math (per row r=(b,t)):
  dyn   = x[:, :31] @ Wd + bd
  value = x[:, 31] * Wv[0] + bv
  xhn   = [dyn | human@Wh+bh | nature@Wn+bn]                      (768 ch)
  pre_j = sum_k cw[t,k] * xhn[perm[ainv[j]+k-3]] + conv_b[t]      (j<256)
  sel   = softmax(relu(pre))
  out   = 0.5*(dyn*sel) @ Wvd1  +  0.5*dyn @ Wvd1 + value @ Wvd2 + bvd
          `------ device ------'  `------- folded into Wdf (host) ------'

Key host-side folds:
  - channel shuffle + depthwise-conv gather -> permuted weight matrix Wg
    [193, 1280] whose column (k*256+j) reproduces xhn[:, perm[ainv[j]+k-3]];
  - all purely-linear output terms -> Wdf (one K=33 matmul);
  - activations are stored PRE-TRANSPOSED (and partition-packed) in DRAM so
    each tile's matmul lhsT loads with ONE contiguous-burst DMA.

The softmax path (sel) only perturbs the output by ~sel (<= a few %), so it
runs in bf16; the linear path stays fp32.

Sharding: pure data parallel, 32 batches per core on 8 cores.
"""

import sys

sys.path.insert(0, "/opt/trn_rl_repo")

from contextlib import ExitStack

import ml_dtypes
import numpy as np

import concourse.bass as bass
import concourse.tile as tile
from concourse import mybir
from concourse.tile import add_dep_helper
from concourse.bass_utils import run_bass_kernel_spmd

# problem constants
B, T, E = 256, 64, 256
XS, DS = 32, 31
HT, NT_ = 80, 80
C = 3 * E
KW = 5
NCORES = 8
BPC = B // NCORES          # 32 batches per core
R = BPC * T                # 2048 rows per core
P = 128
NTILES = R // P            # 16
AK = XS + 1 + HT + NT_     # 193 act rows (transposed): x(32) | ones | h(80) | n(80)
K2 = AK - 128              # 65
NG = KW * E                # 1280 gathered columns

# packed-constants layout (fp32 free-dim offsets in the [128, WPACK] tensor)
O_ID = 0
O_WG1 = O_ID + P            # 128
O_WG2 = O_WG1 + NG          # 1408
O_WDF = O_WG2 + NG          # 2688
O_WV1 = O_WDF + 512         # 3200  (bf16-packed Wvd1: 256 fp32 slots)
O_IDB = O_WV1 + E           # 3456  (bf16 identity: 64 fp32 slots)
O_CW = O_IDB + P // 2       # 3520
O_CB = O_CW + KW            # 3525
WPACK = O_CB + 1            # 3526

F32 = mybir.dt.float32
BF16 = mybir.dt.bfloat16
MULT = mybir.AluOpType.mult
ADD = mybir.AluOpType.add

_NC_CACHE = None
LAST_RESULTS = None
TRACE = False


def _patched_drain_and_barrier(self, tick_clock, wait_clock):
    # The stock kernel-tail drain puts every processor's final-tick wait on a
    # single Drain instruction; this walrus build rejects multi-wait
    # instructions, so spread the waits over a chain of drains instead.
    import bass_rust as _br
    from concourse.vector_clock import ScopedClock

    nc = self.nc
    drain_inst = nc.sync.drain()
    wait_clock.add_sem_waits(
        drain_inst.ins, ScopedClock({None: tick_clock.global_clock})
    )
    si = drain_inst.ins.sync_info
    if si is not None and len(si.on_wait) > 1:
        waits = list(si.on_wait)
        drain_inst.ins.sync_info = _br.SyncInfo(
            on_wait=[waits[0]], on_update=list(si.on_update)
        )
        for w in waits[1:]:
            d2 = nc.sync.drain()
            d2.ins.sync_info = _br.SyncInfo(on_wait=[w], on_update=[])
    nc.all_engine_barrier()
    assert self.sems is not None
    popped = nc._tile_sem_poison_stack.pop()
    assert popped is self._sem_poison
    nc.clear_and_free_semaphores(list(self.sems.allocated().values()))
    nc.all_engine_barrier()


tile.TileContext._drain_and_barrier = _patched_drain_and_barrier


def _build_nc():
    nc = bass.Bass()
    actsP_d = nc.dram_tensor("actsP", [P, 2 * R], F32, kind="ExternalInput")
    wpack_d = nc.dram_tensor("wpack", [P, WPACK], F32, kind="ExternalInput")
    out_d = nc.dram_tensor("out", [R, E], F32, kind="ExternalOutput")
    actsP3 = actsP_d[:, :].rearrange("p (two r) -> p two r", two=2)

    with tile.TileContext(nc) as tc, ExitStack() as ctx:
        consts = ctx.enter_context(tc.tile_pool(name="consts", bufs=1))
        pactT = ctx.enter_context(tc.tile_pool(name="pactT", bufs=16))
        pgs = ctx.enter_context(tc.tile_pool(name="pgs", bufs=2))
        pacc = ctx.enter_context(tc.tile_pool(name="pacc", bufs=3))
        pex = ctx.enter_context(tc.tile_pool(name="pex", bufs=2))
        psml = ctx.enter_context(tc.tile_pool(name="psml", bufs=4))
        pz = ctx.enter_context(tc.tile_pool(name="pz", bufs=2))
        ptch = ctx.enter_context(tc.tile_pool(name="ptch", bufs=2))
        pzT = ctx.enter_context(tc.tile_pool(name="pzT", bufs=2))
        pot = ctx.enter_context(tc.tile_pool(name="pot", bufs=4))
        pG = ctx.enter_context(tc.tile_pool(name="pG", bufs=1, space="PSUM"))
        pdf = ctx.enter_context(tc.tile_pool(name="pdf", bufs=2, space="PSUM"))
        ptz = ctx.enter_context(tc.tile_pool(name="ptz", bufs=2, space="PSUM"))
        pscr = ctx.enter_context(tc.tile_pool(name="pscr", bufs=1, space="PSUM"))

        wp = consts.tile([P, WPACK], F32)
        nc.sync.dma_start(wp[:], wpack_d[:, :])
        ident = wp[:, O_ID : O_ID + P]
        wg1 = wp[:, O_WG1 : O_WG1 + NG]
        wg2 = wp[0:K2, O_WG2 : O_WG2 + NG]
        wdf = wp[0:33, O_WDF : O_WDF + 512]
        wpb = wp[:].bitcast(BF16)
        wv1a = wpb[:, 2 * O_WV1 : 2 * O_WV1 + E]
        wv1b = wpb[:, 2 * O_WV1 + E : 2 * O_WV1 + 2 * E]
        identb = wpb[:, 2 * O_IDB : 2 * O_IDB + P]
        cw = wp[:, O_CW : O_CW + KW]
        cb = wp[:, O_CB : O_CB + 1]

        # PE/DVE observe the weights DMA once (Matmult carries only ONE
        # sem-wait, so it must never still owe this queue a wait).
        scr = pscr.tile([P, 1], F32)
        nc.tensor.matmul(scr[:], ident, cb, start=True, stop=True)
        wtouch = psml.tile([P, 1], F32, tag="sml")
        nc.vector.tensor_copy(wtouch[:], cb)

        z_prev = None
        gs_prev = None
        mmg_last_prev = None
        mmz_prev = None

        for i in range(NTILES):
            rows = slice(i * P, (i + 1) * P)
            # one DMA per tile: [:, 0, :] = act rows 0..127, [0:65, 1, :] =
            # act rows 128..192 (partition-packed by the host)
            actT = pactT.tile([P, 2, P], F32)
            nc.sync.dma_start(actT[:], actsP3[:, :, rows])

            # Matmult instructions can carry only ONE sem-wait.  Tiny bf16
            # ldweights reads act as "PE observes processor X" gadgets: each
            # carries exactly one wait, and every real matmul self-loads its
            # weights, so a stray LDWEIGHTS is harmless.  After these, the
            # matmuls below owe at most their single remaining dependency.
            absorbers = [
                nc.tensor.ldweights(actT[:].bitcast(BF16)[0:1, 0, 0:2]),
            ]
            if z_prev is not None:
                absorbers.append(nc.tensor.ldweights(z_prev[0:1, 0:2]))
            if gs_prev is not None:
                # last columns come from the second ACT drain, whose tick
                # covers both drains of the previous tile
                absorbers.append(nc.tensor.ldweights(gs_prev[0:1, NG - 2 : NG]))
            if mmg_last_prev is not None:
                # PE self-sem: psum-bank write-after-write completion waits
                # can't ride on a Matmult (1-wait limit); park them here
                ldw_self = nc.tensor.ldweights(wpb[0:1, 0:2])
                add_dep_helper(ldw_self.ins, mmg_last_prev.ins, sync=True,
                               reason="absorb PE W-W completion wait")
                if mmz_prev is not None:
                    add_dep_helper(ldw_self.ins, mmz_prev.ins, sync=True,
                                   reason="absorb PE W-W completion wait")
                absorbers.append(ldw_self)

            # dyn (cols 0:256) and folded-linear out part (cols 256:512)
            pdf_t = pdf.tile([P, 512], F32)
            mm_df = nc.tensor.matmul(
                pdf_t[:, :], actT[0:33, 0, :], wdf, start=True, stop=True
            )
            for a in absorbers:
                add_dep_helper(mm_df.ins, a.ins, sync=False,
                               reason="absorbers run before first matmul")

            # gathered conv operand columns
            pG_t = pG.tile([P, NG], F32)
            for s0, s1 in ((0, 512), (512, 1024), (1024, NG)):
                nc.tensor.matmul(
                    pG_t[:, s0:s1], actT[:, 0, :], wg1[:, s0:s1],
                    start=True, stop=False,
                )
                mmg_last_prev = nc.tensor.matmul(
                    pG_t[:, s0:s1], actT[0:K2, 1, :], wg2[:, s0:s1],
                    start=False, stop=True,
                )

            # drain G to SBUF as bf16 on ACT (halves the DVE combine cost)
            gs = pgs.tile([P, NG], BF16)
            nc.scalar.copy(gs[:, 0:640], pG_t[:, 0:640])
            nc.scalar.copy(gs[:, 640:NG], pG_t[:, 640:NG])

            # DVE observes the second ACT drain before the combine chain so
            # no combine op owes more than one foreign-processor wait
            gtouch = ptch.tile([1, 2], BF16, tag="tch")
            nc.vector.tensor_copy(gtouch[:], gs[0:1, NG - 2 : NG])

            # conv combine: pre = sum_k cw[:,k] * G_k (+ conv_b), then relu
            prev = pacc.tile([P, E], BF16, tag="acc")
            nc.vector.tensor_scalar(
                prev[:], gs[:, 0:E], cw[:, 0:1], cb[:, 0:1], op0=MULT, op1=ADD
            )
            for k in range(1, KW):
                nxt = pacc.tile([P, E], BF16, tag="acc")
                nc.vector.scalar_tensor_tensor(
                    nxt[:], gs[:, k * E : (k + 1) * E], cw[:, k : k + 1], prev[:],
                    op0=MULT, op1=ADD,
                )
                prev = nxt
            relu = pacc.tile([P, E], BF16, tag="acc")
            nc.vector.tensor_scalar_max(relu[:], prev[:], 0.0)

            # exp + free row-sum via accum_out
            exm = pex.tile([P, E], BF16, tag="exm")
            ssum = psml.tile([P, 1], F32, tag="sml")
            nc.scalar.activation(
                exm[:], relu[:], func=mybir.ActivationFunctionType.Exp,
                accum_out=ssum[:],
            )
            sinv = psml.tile([P, 1], F32, tag="sml")
            nc.vector.reciprocal(sinv[:], ssum[:])
            shalf = psml.tile([P, 1], F32, tag="sml")
            nc.vector.tensor_scalar_mul(shalf[:], sinv[:], 0.5)

            # DVE observes MM_df's PE tick first, so the gate op only owes
            # its same-engine chain wait
            pdtouch = ptch.tile([1, 2], BF16, tag="tch")
            nc.vector.tensor_copy(pdtouch[:], pdf_t[:].bitcast(BF16)[0:1, 0:2])

            # z = (exm * 0.5/S) * dyn   (bf16; z is a <=6% correction term)
            z = pz.tile([P, E], BF16, tag="z")
            nc.vector.scalar_tensor_tensor(
                z[:], exm[:], shalf[:], pdf_t[:, 0:E], op0=MULT, op1=MULT
            )

            # transpose z on PE, copy to SBUF, fold z @ Wvd1 into out cols
            ptz_t = ptz.tile([P, 2, P], BF16)
            nc.tensor.transpose(ptz_t[:, 0, :], z[:, 0:128], identb)
            nc.tensor.transpose(ptz_t[:, 1, :], z[:, 128:256], identb)
            zT = pzT.tile([P, 2, P], BF16)
            nc.vector.tensor_copy(zT[:], ptz_t[:])
            nc.tensor.matmul(
                pdf_t[:, 256:512], zT[:, 0, :], wv1a,
                start=False, stop=False, skip_group_check=True,
            )
            mmz_prev = nc.tensor.matmul(
                pdf_t[:, 256:512], zT[:, 1, :], wv1b,
                start=False, stop=True, skip_group_check=True,
            )
            z_prev = z
            gs_prev = gs

            # ACT observes the gate's DVE tick before the out-copy
            zt_ = ptch.tile([1, 2], BF16, tag="tch2")
            nc.scalar.copy(zt_[:], z[0:1, 0:2])
            # stage 4 tiles of output, then one large DMA on a fresh queue
            if i % 4 == 0:
                obuf = pot.tile([P, 4, E], F32)
            nc.scalar.copy(obuf[:, i % 4, :], pdf_t[:, 256:512])
            if i % 4 == 3:
                g0 = (i - 3) * P
                odst = out_d[g0 : g0 + 4 * P, :].rearrange(
                    "(t p) e -> p t e", p=P
                )
                nc.gpsimd.dma_start(odst, obuf[:])

    return nc


def _host_prep(x, human, nature, perm, Wv, bv, Wd, bd, Wh, bh, Wn, bn,
               conv_w, conv_b, Wvd, bvd):
    f = np.float32
    x = np.asarray(x, f)
    human = np.asarray(human, f)
    nature = np.asarray(nature, f)
    Wv = np.asarray(Wv, f); bv = np.asarray(bv, f)
    Wd = np.asarray(Wd, f); bd = np.asarray(bd, f)
    Wh = np.asarray(Wh, f); bh = np.asarray(bh, f)
    Wn = np.asarray(Wn, f); bn = np.asarray(bn, f)
    conv_w = np.asarray(conv_w, f)
    conv_b = np.asarray(conv_b, f)
    Wvd = np.asarray(Wvd, f); bvd = np.asarray(bvd, f)
    perm = np.asarray(perm).astype(np.int64)

    Wvd1 = Wvd[:E, :]
    Wvd2 = Wvd[E:, :]

    acts = np.concatenate(
        [
            x.reshape(B * T, XS),
            np.ones((B * T, 1), f),
            human.reshape(B * T, HT),
            nature.reshape(B * T, NT_),
        ],
        axis=1,
    )
    actsT = np.ascontiguousarray(acts.T)  # [193, B*T]
    # partition-packed: [128, 2, B*T]; plane 0 = rows 0..127, plane 1 rows
    # 0..64 = act rows 128..192
    actsP = np.zeros((P, 2, B * T), f)
    actsP[:, 0, :] = actsT[0:128]
    actsP[0:K2, 1, :] = actsT[128:AK]

    wpack = np.zeros((P, WPACK), f)
    wpack[:, O_ID : O_ID + P] = np.eye(P, dtype=f)

    wdf = np.zeros((33, 512), f)
    wdf[0:DS, 0:E] = Wd
    wdf[32, 0:E] = bd
    wdf[0:DS, E:] = 0.5 * (Wd @ Wvd1)
    wdf[31, E:] = Wv[0] @ Wvd2
    wdf[32, E:] = 0.5 * (bd @ Wvd1) + bv @ Wvd2 + bvd
    wpack[0:33, O_WDF : O_WDF + 512] = wdf

    ainv = np.argsort(perm)
    Wg = np.zeros((AK, NG), f)
    for k in range(KW):
        pos = ainv[:E] + k - 3
        for j in range(E):
            pj = pos[j]
            if 0 <= pj < C:
                c = perm[pj]
                col = k * E + j
                if c < E:
                    Wg[0:DS, col] = Wd[:, c]
                    Wg[32, col] = bd[c]
                elif c < 2 * E:
                    Wg[33:113, col] = Wh[:, c - E]
                    Wg[32, col] = bh[c - E]
                else:
                    Wg[113:193, col] = Wn[:, c - 2 * E]
                    Wg[32, col] = bn[c - 2 * E]
    wpack[:, O_WG1 : O_WG1 + NG] = Wg[0:128]
    wpack[0:K2, O_WG2 : O_WG2 + NG] = Wg[128:AK]

    wv1_bf = np.ascontiguousarray(Wvd1.astype(ml_dtypes.bfloat16))  # [256, 256]
    wv1_packed = np.concatenate(
        [wv1_bf[0:128], wv1_bf[128:256]], axis=1
    ).view(np.float32)  # [128, 256]
    wpack[:, O_WV1 : O_WV1 + E] = wv1_packed

    idb = np.ascontiguousarray(np.eye(P, dtype=ml_dtypes.bfloat16))
    wpack[:, O_IDB : O_IDB + P // 2] = idb.view(np.float32)
    wpack[:, O_CW : O_CW + KW] = np.tile(conv_w[:, 0, :], (2, 1))
    wpack[:, O_CB] = np.tile(conv_b, 2)
    return actsP, wpack


def kernel(**inputs):
    global _NC_CACHE, LAST_RESULTS
    actsP, wpack = _host_prep(**inputs)

    if _NC_CACHE is None:
        _NC_CACHE = _build_nc()
    nc = _NC_CACHE

    in_maps = []
    for ci in range(NCORES):
        shard = np.ascontiguousarray(
            actsP[:, :, ci * R : (ci + 1) * R]
        ).reshape(P, 2 * R)
        in_maps.append({"actsP": shard, "wpack": wpack})

    res = run_bass_kernel_spmd(nc, in_maps, core_ids=list(range(NCORES)), trace=TRACE)
    LAST_RESULTS = res

    out = np.empty((B, T, E), np.float32)
    for ci in range(NCORES):
        out[ci * BPC : (ci + 1) * BPC] = res.results[ci]["out"].reshape(BPC, T, E)
    return out
